# revision 22
# baseline (speedup 1.0000x reference)
"""AtomicBasis GNN message-passing kernel for 8 TRN2 NeuronCores (v2).

A[k,x,y,z] = sum_a  c*sin(k*pi*d_a/5)/d_a * (h@W.T)[a,k] * nx*ny*nz,  n = rp/d
           = sum_a  S[a,k] * h'[a,k] * msym[a, sym(x,y,z)]
with S[a,k] = sin(2*pi * k * theta_a), theta = d/10 (turns), and
msym[a,u] = sqrt(2/5)/d^4 * rp_i*rp_j*rp_k for the 10 sorted monomials
(the rank-3 tensor is fully symmetric: 27 entries = 10 unique values).

Host-side marshaling (part of the sharding strategy):
  - h cast to bf16 and pre-transposed into a pair-packed [128, NLOC/2]
    layout so each 128-row step's h^T block [64c, 128a] is DMA-fed
    directly as a matmul stationary (no on-device transposes):
    partitions 0:64 hold k=0..63 for even steps, 64:128 for odd steps.
  - theta (f32) and msym (bf16) precomputed on host in step-major
    layout: partition p of step j owns neighbour a = j*128 + p.
Device per chunk (32 steps = 4096 a):
  gpsimd: arg[p,(i,k)] = theta_fx * k  (int32, theta_fx = round(theta*2^23))
  DVE/gpsimd (alternating): u = (arg & 0x7FFFFF) | 0x4B000000
    -> bitcast f32 = 2^23*(1+frac), so Sin(scale*u - 3*pi) = sin(2*pi*frac - pi)
       = -sin(2*pi*theta*k); the global sign is fixed on the host.
  ACT: S = Sin(scale*u + bias) -> bf16
  PE: h'[128a, 64] = hT_step.T @ W.T per step into PSUM banks (8 steps/bank)
  DVE: w = S * h' (PSUM read) -> bf16 SBUF (one group/chunk goes via a
       ScalarE PSUM->SBUF bf16 copy + DVE bf16 mul to balance engines)
  PE: A^T[10, 64] += msym_step.T @ w_step  (992-matmul accumulation)
Host: sum the 8 cores' A^T partials, negate, expand 10 -> 27 symmetric.
"""

import os
import sys
import numpy as np

for _p in ("/opt/trn_rl_repo", "/root/problem/trn_rl_repo"):
    if os.path.isdir(_p) and _p not in sys.path:
        sys.path.insert(0, _p)

import ml_dtypes

N_GLOBAL = 1_000_000
K = 64
P = 128
NSTEP = 992                  # 128-row steps per core
NLOC = P * NSTEP             # 126976 rows per core
NCORES = 8
NTOT = NCORES * NLOC         # 1015808 >= 1e6 (padded)
T = 32                       # steps per chunk
NCHUNK = NSTEP // T          # 31
GROUP = 16                   # steps per PSUM group (2 banks)
NGRP = T // GROUP            # 2
ASTACK = 4                   # steps stacked per A-matmul (block-diag junk trick)
ARG_DVE = 4                  # steps of arg computed on DVE (rest on gpsimd)
NPAIR = NSTEP // 2           # 496 step-pairs per core
R_CUT = 5.0
C_RBF = float(np.sqrt(2.0 / R_CUT))
TWO_PI_S = 6.2831845         # slightly < 2*pi; |Sin arg| <= pi
FIX = 1 << 23                # fixed-point turns scale
SIN_SCALE = TWO_PI_S / FIX   # u (bitcast f32) = 2^23 * (1 + frac)
SIN_BIAS = -1.5 * TWO_PI_S   # -> arg = 2pi*frac - pi

TRIPLES = [(0, 0, 0), (0, 0, 1), (0, 0, 2), (0, 1, 1), (0, 1, 2),
           (0, 2, 2), (1, 1, 1), (1, 1, 2), (1, 2, 2), (2, 2, 2)]

_CACHE = {}


def _build_nc(NCORES=NCORES):
    import concourse.bass as bass
    import concourse.bacc as bacc
    import concourse.tile as tile
    import concourse.mybir as mybir

    f32 = mybir.dt.float32
    bf16 = mybir.dt.bfloat16
    i32 = mybir.dt.int32

    nc = bacc.Bacc(
        "TRN2",
        target_bir_lowering=False,
        debug=False,
        enable_asserts=True,
        num_devices=NCORES,
    )

    ht_ext = nc.dram_tensor("ht", [P, NPAIR * P], bf16, kind="ExternalInput").ap()
    th_ext = nc.dram_tensor("th", [P, NSTEP], i32, kind="ExternalInput").ap()
    ms_ext = nc.dram_tensor("ms", [P, NSTEP * 10], bf16, kind="ExternalInput").ap()
    wt_ext = nc.dram_tensor("wt", [P, 2 * K], bf16, kind="ExternalInput").ap()
    io_ext = nc.dram_tensor("iota1", [P, K], i32, kind="ExternalInput").ap()
    out_ext = nc.dram_tensor(
        "out", [10 * ASTACK, K * ASTACK], f32, kind="ExternalOutput"
    ).ap()

    SIN = mybir.ActivationFunctionType.Sin
    BAND = mybir.AluOpType.bitwise_and
    BOR = mybir.AluOpType.bitwise_or

    with tile.TileContext(nc) as tc:
        from contextlib import ExitStack

        with ExitStack() as ctx:
            const = ctx.enter_context(tc.tile_pool(name="const", bufs=1))
            hpool = ctx.enter_context(tc.tile_pool(name="hch", bufs=3))
            mpool = ctx.enter_context(tc.tile_pool(name="mch", bufs=3))
            argp = ctx.enter_context(tc.tile_pool(name="argp", bufs=2))
            up = ctx.enter_context(tc.tile_pool(name="up", bufs=2))
            sp = ctx.enter_context(tc.tile_pool(name="sp", bufs=2))
            wp = ctx.enter_context(tc.tile_pool(name="wp", bufs=2))
            psH = ctx.enter_context(
                tc.tile_pool(name="psH", bufs=2, space=bass.MemorySpace.PSUM)
            )
            psA = ctx.enter_context(
                tc.tile_pool(name="psA", bufs=1, space=bass.MemorySpace.PSUM)
            )

            # ---------------- prologue ----------------
            th_all = const.tile([P, NSTEP], i32)
            nc.sync.dma_start(th_all[:], th_ext)
            wt_sb = const.tile([P, 2 * K], bf16)
            nc.sync.dma_start(wt_sb[:], wt_ext)
            io_sb = const.tile([P, K], i32)
            nc.sync.dma_start(io_sb[:], io_ext)
            sbias = const.tile([P, 1], f32)
            nc.vector.memset(sbias[:], SIN_BIAS)

            A_ps = psA.tile([10 * ASTACK, K * ASTACK], f32)
            NQUAD = GROUP // ASTACK      # A-matmuls per group
            LASTQ = (NCHUNK - 1, NGRP - 1, NQUAD - 1)

            # ---------------- main loop ----------------
            for c in range(NCHUNK):
                h_ch = hpool.tile([P, (T // 2) * P], bf16)
                nc.sync.dma_start(h_ch[:], ht_ext[:, c * 2048 : (c + 1) * 2048])
                ms_ch = mpool.tile([P, T * 10], bf16)
                nc.sync.dma_start(
                    ms_ch[:], ms_ext[:, c * T * 10 : (c + 1) * T * 10]
                )

                arg = argp.tile([P, T * K], i32)
                split = (T - ARG_DVE) * K
                th_sl = th_all[:, c * T : (c + 1) * T]
                nc.gpsimd.tensor_mul(
                    arg[:, 0:split].rearrange("p (i k) -> p i k", i=T - ARG_DVE),
                    th_sl[:, 0 : T - ARG_DVE].unsqueeze(2).broadcast_to((P, T - ARG_DVE, K)),
                    io_sb[:].unsqueeze(1).broadcast_to((P, T - ARG_DVE, K)),
                )
                nc.vector.tensor_mul(
                    arg[:, split:].rearrange("p (i k) -> p i k", i=ARG_DVE),
                    th_sl[:, T - ARG_DVE : T].unsqueeze(2).broadcast_to((P, ARG_DVE, K)),
                    io_sb[:].unsqueeze(1).broadcast_to((P, ARG_DVE, K)),
                )
                u = up.tile([P, T * K], i32)
                nc.vector.tensor_scalar(u[:], arg[:], 0x7FFFFF, 0x4B000000, BAND, BOR)
                s_bf = sp.tile([P, T * K], bf16)
                nc.scalar.activation(
                    s_bf[:], u[:].bitcast(f32), SIN, bias=sbias[:], scale=SIN_SCALE
                )

                for g in range(NGRP):
                    hb = psH.tile([P, GROUP * K], f32)
                    for jp in range(GROUP // 2):
                        pair = g * (GROUP // 2) + jp
                        # one matmul covers the (even, odd) step pair:
                        # moving = [[W^T, 0], [0, W^T]] -> out [h'_even | h'_odd]
                        nc.tensor.matmul(
                            hb[:, jp * P : (jp + 1) * P],
                            h_ch[:, pair * P : (pair + 1) * P],
                            wt_sb[:],
                            start=True,
                            stop=True,
                            skip_group_check=True,
                        )
                    w_bf = wp.tile([P, GROUP * K], bf16)
                    s_sl = s_bf[:, g * GROUP * K : (g + 1) * GROUP * K]
                    if g == 0:
                        # balance: ScalarE copies PSUM->SBUF, DVE does bf16 mul
                        hcp = wp.tile([P, GROUP * K], bf16, tag="hcp")
                        nc.scalar.copy(hcp[:], hb[:])
                        nc.vector.tensor_mul(w_bf[:], s_sl, hcp[:])
                    else:
                        nc.vector.tensor_mul(w_bf[:], s_sl, hb[:])
                    # A-matmuls: ASTACK steps per instruction; the stationary
                    # stacks ASTACK steps' msym cols, moving stacks their w
                    # cols; off-diagonal blocks accumulate junk we drop later.
                    for q in range(NQUAD):
                        s0 = g * GROUP + q * ASTACK
                        nc.tensor.matmul(
                            A_ps[:],
                            ms_ch[:, s0 * 10 : (s0 + ASTACK) * 10],
                            w_bf[:, (q * ASTACK) * K : (q + 1) * ASTACK * K],
                            start=(c == 0 and g == 0 and q == 0),
                            stop=((c, g, q) == LASTQ),
                            skip_group_check=True,
                        )

            # ---------------- epilogue ----------------
            A_sb = const.tile([10 * ASTACK, K * ASTACK], f32)
            nc.vector.tensor_copy(A_sb[:], A_ps[:])
            nc.sync.dma_start(out_ext, A_sb[:])

    nc.compile()
    return nc


def _get_nc():
    if "nc" not in _CACHE:
        _CACHE["nc"] = _build_nc()
    return _CACHE["nc"]


def _marshal(h, rel_poss, W):
    bf16 = ml_dtypes.bfloat16
    h_bf = np.zeros((NTOT, K), dtype=bf16)
    h_bf[:N_GLOBAL] = h.astype(bf16)

    rp = np.asarray(rel_poss, dtype=np.float32)
    d2 = rp[0] * rp[0] + rp[1] * rp[1] + rp[2] * rp[2]
    d = np.sqrt(d2)
    th = np.zeros(NTOT, dtype=np.int32)
    th[:N_GLOBAL] = np.round(
        d.astype(np.float64) * (FIX / (2.0 * R_CUT))
    ).astype(np.int32)
    base = np.float32(C_RBF) / (d2 * d2)
    ms = np.zeros((NTOT, 10), dtype=bf16)
    for uu, (i, j, k) in enumerate(TRIPLES):
        ms[:N_GLOBAL, uu] = (rp[i] * rp[j] * rp[k] * base).astype(bf16)

    wt1 = W.T.astype(bf16)
    wt = np.zeros((P, 2 * K), dtype=bf16)      # block-diag [[W^T, 0], [0, W^T]]
    wt[0:K, 0:K] = wt1
    wt[K : 2 * K, K : 2 * K] = wt1
    iota1 = np.ascontiguousarray(
        np.broadcast_to(np.arange(1, K + 1, dtype=np.int32), (P, K))
    )

    in_maps = []
    for i in range(NCORES):
        sl = slice(i * NLOC, (i + 1) * NLOC)
        ht_i = np.ascontiguousarray(
            h_bf[sl].reshape(NPAIR, 2, P, K).transpose(1, 3, 0, 2).reshape(P, NPAIR * P)
        )
        th_i = np.ascontiguousarray(th[sl].reshape(NSTEP, P).T)
        ms_i = np.ascontiguousarray(
            ms[sl].reshape(NSTEP, P, 10).transpose(1, 0, 2).reshape(P, NSTEP * 10)
        )
        in_maps.append(
            {"ht": ht_i, "th": th_i, "ms": ms_i, "wt": wt, "iota1": iota1}
        )
    return in_maps


def kernel(h, rel_poss, W):
    from concourse.bass_utils import run_bass_kernel_spmd

    nc = _get_nc()
    in_maps = _marshal(h, rel_poss, W)

    res = run_bass_kernel_spmd(
        nc, in_maps, core_ids=list(range(NCORES)), trace=_CACHE.get("trace", False)
    )
    _CACHE["last_results"] = res
    A4 = -np.sum(
        [np.asarray(res.results[i]["out"], dtype=np.float32) for i in range(NCORES)],
        axis=0,
    )  # [10*ASTACK, K*ASTACK]; negated: device computes -sin (half-turn offset)
    AsymT = np.zeros((10, K), dtype=np.float32)
    for j in range(ASTACK):
        AsymT += A4[j * 10 : (j + 1) * 10, j * K : (j + 1) * K]
    A = np.empty((K, 27), dtype=np.float32)
    col = 0
    for x in range(3):
        for y in range(3):
            for z in range(3):
                uu = TRIPLES.index(tuple(sorted((x, y, z))))
                A[:, col] = AsymT[uu]
                col += 1
    return A.reshape(K, 3, 3, 3)


if __name__ == "__main__":
    nc = _get_nc()
    print("build + compile OK")
